# revision 33
# baseline (speedup 1.0000x reference)
"""Trainium2 Bass kernel for nn_CombinedLoss_781684048617.

Pure data parallel over 8 NeuronCores (B=262144 rows -> 8 x 32768); only a
[80, 206] partial-sum gram leaves each core.  The loss reduces to global sums
that the PE picks up as two PSUM-accumulated grams over every 128-row block k,
with y_true's logit columns (exact 0/1 in fp8) as the stationary operand:

  psA[80,126] += yt_logits_k^T @ R_k   R = [min(d^2,1) | relu(d-1) | -relu(-d-1)
                                            (as min(d+1,0)) | lse | 1]
  psB[80, 80] += yt_logits_k^T @ yp_logits_k

Row 16e+c of either gram is the masked sum over slot-e rows of class c, so the
host recovers: active counts (ones col), the CE pieces (lse col, psB diag /
block sums), and masked SmoothL1 via  sl1 = 0.5*min(d^2,1) + relu(d-1) +
relu(-d-1)  -- three columns whose per-class masking (j < num_params[c]) and
0.5/+1/-1 coefficients are applied host-side, which removes the slow
scalar_tensor_tensor and all abs/relu chains from the device.

Inputs stream HBM->SBUF through gpsimd (SWDGE) DMAs that cast fp32->fp8e3m4
in flight (DMA cost is SBUF-side bytes; 4 mantissa bits keep N(0,1) logits /
[0,1) params to ~1% and one-hots exact).  DVE work per tile is a 2x-mode
add-tree for the softmax denominator plus one sub and three 4x-mode fused
tensor_scalar ops; ACT does exp/ln/square from one preloaded table set.  The
unmasked-SmoothL1 fallback is dead code (num_params_per_effect >= 1 forces
param_count >= mask_count), so no accumulator pass is needed.
"""

import sys

import numpy as np

if "/opt/trn_rl_repo" not in sys.path:
    sys.path.insert(0, "/opt/trn_rl_repo")

# ---- problem constants (hardcoded per contract) ----
B_FULL = 262144
NCORES = 8
N_CORE = B_FULL // NCORES  # 32768
E, C, P, ITEM = 5, 16, 8, 24
D = E * ITEM  # 120
LS = 0.05
REG_W = 1.0

# ---- kernel tiling ----
PARTS = 128
# big tiles early (amortize SWDGE fixed cost), small tiles at the end
# (short post-last-DMA compute tail); rows = 128 * sum(TILES) = 32768
TILES = [24, 40, 48, 48, 48, 32, 16]
SQ_DVE_TILES = (2,)  # tiles whose d^2 runs on DVE instead of ACT (balance knob)
EP = E * P  # 40 param cells per row
NLOG = E * C  # 80 logit cells per row
# R (moving gram operand) columns
COL_M2 = 0  # + 8e+j : min(d^2, 1)
COL_TP = EP  # + 8e+j : max(d-1, 0)
COL_UM = 2 * EP  # + 8e+j : min(d+1, 0)  (enters sl1 with weight -1)
COL_LSE = 3 * EP  # + e  : ln(sum_c exp(logit))
COL_ONE = 3 * EP + E  # ones
RW = COL_ONE + 1  # 126
GW = RW + D  # host gram width: [psA(126) | psB(120)]

_CACHE = {}


def _build_bass():
    from contextlib import ExitStack

    import concourse.bacc as bacc
    import concourse.bass as bass
    import concourse.tile as tile
    from concourse import mybir

    assert PARTS * sum(TILES) == N_CORE

    f32 = mybir.dt.float32
    f16 = mybir.dt.float16
    f8 = mybir.dt.float8e3  # e3m4: 4 mantissa bits, ~1% rel err, 0/1 exact
    AF = mybir.ActivationFunctionType
    OP = mybir.AluOpType

    nc = bacc.Bacc(None, target_bir_lowering=False)
    yp_d = nc.dram_tensor("y_pred", [N_CORE, D], f32, kind="ExternalInput")
    yt_d = nc.dram_tensor("y_true", [N_CORE, D], f32, kind="ExternalInput")
    out_g = nc.dram_tensor("out_g", [D, GW], f32, kind="ExternalOutput")

    with tile.TileContext(nc) as tc, ExitStack() as ctx:
        inp = ctx.enter_context(tc.tile_pool(name="inp", bufs=8))
        work = ctx.enter_context(tc.tile_pool(name="work", bufs=3))
        rpool = ctx.enter_context(tc.tile_pool(name="rpool", bufs=4))
        singles = ctx.enter_context(tc.tile_pool(name="singles", bufs=1))
        psum = ctx.enter_context(
            tc.tile_pool(name="psum", bufs=1, space=bass.MemorySpace.PSUM)
        )

        # stationary APs must be single-free-dim, so the full 120-wide y_true
        # row is the stationary operand; its param rows of psA/psB are unused
        psA = psum.tile([D, RW], f32)
        psB = psum.tile([D, D], f32)

        NT = len(TILES)

        def _emit_tree_dve(prev):
            j, KT, yt_t, R_t, ex_t = prev
            h8 = work.tile([PARTS, KT, E, 8], f16)
            nc.vector.tensor_add(h8, ex_t[:, :, :, 0:8], ex_t[:, :, :, 8:16])
            h4 = work.tile([PARTS, KT, E, 4], f16)
            nc.vector.tensor_add(h4, h8[:, :, :, 0:4], h8[:, :, :, 4:8])
            return h4

        row0 = 0
        for i, KT in enumerate(TILES):
            ypv = yp_d[row0 : row0 + PARTS * KT].rearrange(
                "(p k) f -> p k f", k=KT
            )
            ytv = yt_d[row0 : row0 + PARTS * KT].rearrange(
                "(p k) f -> p k f", k=KT
            )
            row0 += PARTS * KT
            yp_t = inp.tile([PARTS, KT, D], f8)
            yt_t = inp.tile([PARTS, KT, D], f8)
            nc.gpsimd.dma_start(out=yp_t, in_=ypv)
            nc.gpsimd.dma_start(out=yt_t, in_=ytv)

            yp4 = yp_t.rearrange("p k (e i) -> p k e i", i=ITEM)
            yt4 = yt_t.rearrange("p k (e i) -> p k e i", i=ITEM)

            R_t = rpool.tile([PARTS, KT, RW], f16)
            Rm2 = R_t[:, :, COL_M2:COL_TP].rearrange("p k (e j) -> p k e j", j=P)
            Rtp = R_t[:, :, COL_TP:COL_UM].rearrange("p k (e j) -> p k e j", j=P)
            Rum = R_t[:, :, COL_UM:COL_LSE].rearrange("p k (e j) -> p k e j", j=P)
            nc.gpsimd.memset(R_t[:, :, COL_ONE : COL_ONE + 1], 1.0)

            # tile 0 only: sub needs just the DMAs — putting it first lets
            # DVE start ~1.5us before the first exp lands
            d_t = work.tile([PARTS, KT, E, P], f16)
            if i == 0:
                nc.vector.tensor_sub(
                    d_t, yp4[:, :, :, C:ITEM], yt4[:, :, :, C:ITEM]
                )

            # tile i: exp + add tree (big levels DVE 2x-mode, small on Pool)
            ex_t = work.tile([PARTS, KT, E, C], f16)
            nc.scalar.activation(out=ex_t, in_=yp4[:, :, :, 0:C], func=AF.Exp)
            h4p = _emit_tree_dve((i, KT, yt_t, R_t, ex_t))
            h2 = work.tile([PARTS, KT, E, 2], f16)
            nc.gpsimd.tensor_add(h2, h4p[:, :, :, 0:2], h4p[:, :, :, 2:4])
            s_t = work.tile([PARTS, KT, E], f16)
            nc.gpsimd.tensor_add(s_t, h2[:, :, :, 0], h2[:, :, :, 1])
            nc.scalar.activation(
                out=R_t[:, :, COL_LSE : COL_LSE + E], in_=s_t, func=AF.Ln
            )

            # reg chain
            if i != 0:
                nc.vector.tensor_sub(
                    d_t, yp4[:, :, :, C:ITEM], yt4[:, :, :, C:ITEM]
                )
            sq_t = work.tile([PARTS, KT, E, P], f16)
            if i in SQ_DVE_TILES:
                nc.vector.tensor_mul(sq_t, d_t, d_t)
            else:
                nc.scalar.activation(out=sq_t, in_=d_t, func=AF.Square)
            nc.vector.tensor_scalar(
                out=Rm2, in0=sq_t, scalar1=1.0, scalar2=None, op0=OP.min
            )
            nc.vector.tensor_scalar(
                out=Rtp, in0=d_t, scalar1=-1.0, scalar2=0.0, op0=OP.add, op1=OP.max
            )
            nc.vector.tensor_scalar(
                out=Rum, in0=d_t, scalar1=1.0, scalar2=0.0, op0=OP.add, op1=OP.min
            )

            # psB(i): needs only the tile-i DMAs; issued before psA so PE
            # overlaps the R compute chain
            for k in range(KT):
                nc.tensor.matmul(
                    psB,
                    yt_t[:, k, :],
                    yp_t[:, k, :],
                    start=i == 0 and k == 0,
                    stop=i == NT - 1 and k == KT - 1,
                )
            for k in range(KT):
                nc.tensor.matmul(
                    psA,
                    yt_t[:, k, :],
                    R_t[:, k, :],
                    start=i == 0 and k == 0,
                    stop=i == NT - 1 and k == KT - 1,
                )

        stage = singles.tile([D, GW], f32)
        nc.vector.tensor_scalar(
            out=stage[:, 0:RW], in0=psA, scalar1=0.0, scalar2=None, op0=OP.add
        )
        nc.vector.tensor_scalar(
            out=stage[:, RW:GW], in0=psB, scalar1=0.0, scalar2=None, op0=OP.add
        )
        nc.sync.dma_start(out=out_g[:], in_=stage)

    # Pre-load the one ACT table set covering Exp/Ln/Square so the greedy
    # per-activation selector never reloads tables mid-loop.
    from concourse.hw_specs import get_activation_tables

    tables = list(get_activation_tables(nc.m.arch).items())
    set_id = next(
        i for i, (name, _) in enumerate(tables)
        if name == "natural_log_exp_and_others"
    )
    load = mybir.InstLoadActFuncSet(
        name=nc.get_next_instruction_name(), act_func_set_id=set_id, ins=[], outs=[]
    )
    load.engine = mybir.EngineType.Activation
    nc.register_instruction(load)
    placed = False
    for blk in nc.m.functions[0].blocks:
        for idx, inst in enumerate(blk.instructions):
            if isinstance(inst, mybir.InstActivation):
                blk.instructions.insert(idx, load)
                placed = True
                break
        if placed:
            break
    assert placed

    nc.compile()
    return nc


def _get_nc():
    if "nc" not in _CACHE:
        _CACHE["nc"] = _build_bass()
    return _CACHE["nc"]


def kernel(y_pred, y_true, num_params_per_effect):
    from concourse.bass_utils import run_bass_kernel_spmd

    yp = np.ascontiguousarray(np.asarray(y_pred, dtype=np.float32))
    yt = np.ascontiguousarray(np.asarray(y_true, dtype=np.float32))
    npf = np.asarray(num_params_per_effect, dtype=np.int64)

    yp_sh = yp.reshape(NCORES, N_CORE, D)
    yt_sh = yt.reshape(NCORES, N_CORE, D)
    in_maps = [
        {"y_pred": yp_sh[i], "y_true": yt_sh[i]} for i in range(NCORES)
    ]

    nc = _get_nc()
    results = run_bass_kernel_spmd(nc, in_maps, list(range(NCORES))).results

    # ---- host-side scalar assembly in float64 ----
    G = np.zeros((D, GW), np.float64)
    for res in results:
        G += np.asarray(res["out_g"], np.float64)
    A = G[:, 0:RW]
    Bm = G[:, RW:GW]

    Tmask = (np.arange(P)[None, :] < npf[:, None]).astype(np.float64)  # [C,P]
    # masked sl1 per (row 24e+c, col 8e+j); coefficients applied here
    SL = 0.5 * A[:, COL_M2:COL_TP] + A[:, COL_TP:COL_UM] - A[:, COL_UM:COL_LSE]

    MSUM = 0.0
    PCNT = 0.0
    LSEt = 0.0
    DX = 0.0
    AFSX = 0.0
    RSUM = 0.0
    for e in range(E):
        rows = slice(ITEM * e, ITEM * e + C)  # y_true logit rows of slot e
        cnt = A[rows, COL_ONE]  # per-class active counts [C]
        MSUM += cnt.sum()
        PCNT += (npf * cnt).sum()
        LSEt += A[rows, COL_LSE + e].sum()
        blk = Bm[rows, ITEM * e : ITEM * e + C]
        DX += np.trace(blk)
        AFSX += blk.sum()
        RSUM += (Tmask * SL[rows, P * e : P * (e + 1)]).sum()

    CSUM = LSEt - (1.0 - LS) * DX - (LS / C) * AFSX

    loss_cls = CSUM / max(MSUM, 1.0) if MSUM > 0 else 0.0
    # num_params_per_effect >= 1 makes PCNT >= MSUM, so the unmasked
    # fallback of the reference is unreachable whenever MSUM > 0
    loss_reg = (RSUM / max(PCNT, 1.0)) if MSUM > 0 else 0.0
    total = loss_cls + REG_W * loss_reg

    return (
        np.float32(total),
        np.float32(loss_cls),
        np.float32(loss_reg),
    )
